# revision 1
# baseline (speedup 1.0000x reference)
"""Bass/Trainium2 kernel for nn_HadamardClassifier.

Math: out = -scale * l2norm(x) @ H + bias, with H = H_16384[:2048, :14951]
(Sylvester). Since H_16384 = H_8 (x) H_2048 and rows < 2048 hit only row 0 of
the H_8 factor (all +1), H is just H_2048 tiled horizontally:
    out[:, j] = (x * (-scale/||x||_2)) @ H_2048[:, j % 2048] + bias[j]

Sharding: batch-parallel across 8 cores (512 rows each).

Numerics: H_2048 entries are exactly +-1 so it is cast to bf16 losslessly and
the matmul runs on the PE at bf16 rate (2x fp32). x is split into bf16 hi+lo
parts (x ~= xh + xl) and both pass through the same accumulation group, so
Z = xh@H + xl@H matches the f32 product to ~1e-6 relative.
"""

import math

import numpy as np

B, IN, OUT = 4096, 2048, 14951
NCORES = 8
BLOC = B // NCORES  # 512
P = 128
PERIOD = 2048
NFULL = 7  # full 2048-wide output blocks
TAIL = OUT - NFULL * PERIOD  # 615
EPS = 1e-12
NCB = BLOC // P  # 4 batch chunks per core
NIC = IN // P  # 16 contraction chunks
NJS = PERIOD // 512  # 4 column slabs of 512
# bias packed js-major: per js, its 512-col piece of every block, 512-padded
NBLK_JS = [8, 8, 7, 7]  # js1's 8th seg is the 103-wide tail (padded)
OFF_JS = [0, 4096, 8192, 11776]
BIAS_PACK = 15360

_CACHE = {}
LAST_RESULT = None
PROFILE = False


def _build(scale_val: float):
    from contextlib import ExitStack

    import concourse.bass as bass
    import concourse.mybir as mybir
    import concourse.tile as tile
    from concourse import bacc, masks

    f32 = mybir.dt.float32
    bf16 = mybir.dt.bfloat16
    nc = bacc.Bacc("TRN2", target_bir_lowering=False, debug=False,
                   num_devices=NCORES)

    x_d = nc.dram_tensor("x", [BLOC, IN], f32, kind="ExternalInput")
    h_d = nc.dram_tensor("h", [IN, PERIOD], bf16, kind="ExternalInput")
    b_d = nc.dram_tensor("bias", [1, BIAS_PACK], f32, kind="ExternalInput")
    o_d = nc.dram_tensor("out", [BLOC, OUT], f32, kind="ExternalOutput")

    # [2048 rows] -> [p, ic] view so each SBUF partition p holds rows ic*128+p
    h_v = h_d[:, :].rearrange("(ic p) j -> p ic j", p=P)
    # main output region as [rows, blk, col-in-block]
    o_main = o_d[:, 0 : NFULL * PERIOD].rearrange("r (blk c) -> r blk c",
                                                  c=PERIOD)

    with tile.TileContext(nc) as tc, ExitStack() as ctx:
        p_const = ctx.enter_context(tc.tile_pool(name="const", bufs=1))
        p_x = ctx.enter_context(tc.tile_pool(name="xload", bufs=2))
        p_w = ctx.enter_context(tc.tile_pool(name="work", bufs=1))
        p_ss = ctx.enter_context(tc.tile_pool(name="small", bufs=8))
        p_xth = ctx.enter_context(tc.tile_pool(name="xth", bufs=NCB))
        p_xtl = ctx.enter_context(tc.tile_pool(name="xtl", bufs=NCB))
        p_h = ctx.enter_context(tc.tile_pool(name="hslab", bufs=6))
        p_z = ctx.enter_context(tc.tile_pool(name="zsb", bufs=4))
        p_o = ctx.enter_context(tc.tile_pool(name="ostage", bufs=4))
        p_pst = ctx.enter_context(
            tc.tile_pool(name="psum_t", bufs=2, space="PSUM"))
        p_psz = ctx.enter_context(
            tc.tile_pool(name="psum_z", bufs=6, space="PSUM"))

        ident = p_const.tile([P, P], f32, tag="ident")
        masks.make_identity(nc, ident[:])

        # HAM warmup: keep the PE busy early so the clock gate opens
        # (4/8 -> 8/8) before the real matmul stream starts
        warm = p_pst.tile([P, P], f32, tag="pst")
        for _ in range(16):
            nc.tensor.matmul(warm[:], ident[:], ident[:], start=True,
                             stop=True)

        # bias: load packed row into partition 0; broadcast in per-block
        # chunks on gpsimd, interleaved into phase 2 so the SWDGE out-DMA
        # queue is never blocked long
        bias_rep = p_const.tile([P, BIAS_PACK], f32, tag="bias_rep")
        nc.sync.dma_start(out=bias_rep[0:1, :], in_=b_d[:, :])

        def bcast_js(js):
            o0 = OFF_JS[js]
            for blk in range(NBLK_JS[js]):
                a = o0 + blk * 512
                nc.gpsimd.partition_broadcast(bias_rep[:, a : a + 512],
                                              bias_rep[0:1, a : a + 512])

        bcast_js(0)
        # remaining 22 chunks dribbled out a few per iteration (deadline:
        # js's first add), so the SWDGE out-DMA queue never blocks long
        bcast_rest = [(js, blk) for js in range(1, NJS)
                      for blk in range(NBLK_JS[js])]

        # ---- phase 1: load x, l2-normalize rows (folding -scale),
        # transpose, split into bf16 hi+lo
        xths, xtls = [], []
        hq_tiles = {}
        for cb in range(NCB):
            xnat = p_x.tile([P, IN], f32, tag="xnat")
            nc.sync.dma_start(out=xnat[:], in_=x_d[cb * P : (cb + 1) * P, :])
            # interleave js0/js1's H halves into the load queue
            hq = p_h.tile([P, 8, 512], bf16, tag="hslab")
            half = cb % 2
            js = cb // 2
            nc.sync.dma_start(
                out=hq[:],
                in_=h_v[:, half * 8 : half * 8 + 8, js * 512 : js * 512 + 512])
            hq_tiles[(js, half)] = hq

            sq = p_w.tile([P, IN], bf16, tag="work")
            ss = p_ss.tile([P, 1], f32, tag="ss")
            nc.scalar.activation(sq[:], xnat[:],
                                 mybir.ActivationFunctionType.Square,
                                 accum_out=ss[:])
            nc.vector.tensor_scalar_max(ss[:], ss[:], EPS)
            nrm = p_ss.tile([P, 1], f32, tag="nrm")
            nc.scalar.sqrt(nrm[:], ss[:])
            inv = p_ss.tile([P, 1], f32, tag="inv")
            nc.vector.reciprocal(inv[:], nrm[:])
            mult = p_ss.tile([P, 1], f32, tag="mult")
            nc.vector.tensor_scalar_mul(mult[:], inv[:], -scale_val)

            # scale rows in place on ACT (DVE is the busier engine)
            nc.scalar.mul(xnat[:], xnat[:], mult[:, 0:1])

            xth = p_xth.tile([P, NIC, P], bf16, tag="xth")
            xtl = p_xtl.tile([P, NIC, P], bf16, tag="xtl")
            for ic in range(NIC):
                pst = p_pst.tile([P, P], f32, tag="pst")
                nc.tensor.transpose(pst[:], xnat[:, ic * P : (ic + 1) * P],
                                    ident[:])
                # hi = bf16(xn^T); lo = bf16(xn^T - hi)
                nc.scalar.copy(xth[:, ic, :], pst[:])
                nc.vector.tensor_sub(xtl[:, ic, :], pst[:], xth[:, ic, :])
            xths.append(xth)
            xtls.append(xtl)

        # ---- hoist remaining H loads (slot waits throttle them)
        for js in range(NJS):
            for half in range(2):
                if (js, half) not in hq_tiles:
                    hq = p_h.tile([P, 8, 512], bf16, tag="hslab")
                    nc.sync.dma_start(
                        out=hq[:],
                        in_=h_v[:, half * 8 : half * 8 + 8,
                                js * 512 : js * 512 + 512])
                    hq_tiles[(js, half)] = hq

        # ---- phase 2: Z = xn' @ H_2048 slab by slab; add bias; store
        for js in range(NJS):
            c0 = js * 512
            boff = OFF_JS[js]
            for cb in range(NCB):
                last = (js == NJS - 1 and cb == NCB - 1)
                # the very last iteration runs in two 256-col halves so the
                # post-matmul adds+store tail is half as long
                col_chunks = [(0, 256), (256, 256)] if last else [(0, 512)]
                psz = p_psz.tile([P, 512], f32, tag="psz")
                for ic in range(NIC):
                    hap = hq_tiles[(js, ic // 8)][:, ic % 8, :]
                    nc.tensor.matmul(psz[:], xths[cb][:, ic, :], hap,
                                     start=(ic == 0), stop=False)
                    nc.tensor.matmul(psz[:], xtls[cb][:, ic, :], hap,
                                     start=False, stop=(ic == NIC - 1))
                for w0, wn in col_chunks:
                    zsb = p_z.tile([P, 512], f32, tag="zsb")
                    nc.scalar.copy(zsb[:, 0:wn], psz[:, w0 : w0 + wn])

                    ost1 = p_o.tile([P, 4, 512], f32, tag="ostage")
                    ost2 = p_o.tile([P, 4, 512], f32, tag="ostage")
                    for blk in range(4):
                        nc.vector.tensor_add(
                            ost1[:, blk, 0:wn], zsb[:, 0:wn],
                            bias_rep[:, boff + blk * 512 + w0 :
                                     boff + blk * 512 + w0 + wn])
                    for blk in range(4, 7):
                        nc.vector.tensor_add(
                            ost2[:, blk - 4, 0:wn], zsb[:, 0:wn],
                            bias_rep[:, boff + blk * 512 + w0 :
                                     boff + blk * 512 + w0 + wn])
                    tw = 512 if js == 0 else (103 if js == 1 else 0)
                    tw = max(min(tw - w0, wn), 0)
                    if tw:
                        nc.vector.tensor_add(
                            ost2[:, 3, 0:tw], zsb[:, 0:tw],
                            bias_rep[:, boff + 7 * 512 + w0 :
                                     boff + 7 * 512 + w0 + tw])

                    r0 = cb * P
                    nc.gpsimd.dma_start(
                        out=o_main[r0 : r0 + P, 0:4,
                                   c0 + w0 : c0 + w0 + wn],
                        in_=ost1[:, :, 0:wn])
                    nc.gpsimd.dma_start(
                        out=o_main[r0 : r0 + P, 4:7,
                                   c0 + w0 : c0 + w0 + wn],
                        in_=ost2[:, 0:3, 0:wn])
                    if tw:
                        nc.gpsimd.dma_start(
                            out=o_d[r0 : r0 + P, NFULL * PERIOD + c0 + w0 :
                                    NFULL * PERIOD + c0 + w0 + tw],
                            in_=ost2[:, 3, 0:tw])
                # stagger the remaining bias broadcasts behind out-DMAs
                for _ in range(3):
                    if bcast_rest:
                        bjs, bblk = bcast_rest.pop(0)
                        a = OFF_JS[bjs] + bblk * 512
                        nc.gpsimd.partition_broadcast(
                            bias_rep[:, a : a + 512], bias_rep[0:1, a : a + 512])

    nc.compile()
    return nc


def _pack_bias(bias: np.ndarray) -> np.ndarray:
    pack = np.zeros((1, BIAS_PACK), dtype=np.float32)
    for js in range(NJS):
        for blk in range(NBLK_JS[js]):
            src0 = blk * PERIOD + js * 512
            seg = bias[src0 : src0 + 512]
            pack[0, OFF_JS[js] + blk * 512 : OFF_JS[js] + blk * 512 + len(seg)] = seg
    return pack


def kernel(x, hadamard, scale, bias):
    global LAST_RESULT
    import ml_dtypes
    from concourse.bass_utils import run_bass_kernel_spmd

    x = np.ascontiguousarray(np.asarray(x, dtype=np.float32))
    hadamard = np.asarray(hadamard, dtype=np.float32)
    bias = np.asarray(bias, dtype=np.float32)
    scale_val = float(np.asarray(scale).reshape(-1)[0])

    h2 = np.ascontiguousarray(hadamard[:, :PERIOD])
    # the whole kernel rests on the 2048-periodicity of the weight columns
    for k in range(1, NFULL):
        assert np.array_equal(hadamard[:, k * PERIOD : (k + 1) * PERIOD], h2), (
            "hadamard is not 2048-periodic; kernel assumption violated")
    assert np.array_equal(hadamard[:, NFULL * PERIOD :], h2[:, :TAIL])
    h2b = h2.astype(ml_dtypes.bfloat16)
    assert np.array_equal(h2b.astype(np.float32), h2), "H not bf16-exact"

    key = scale_val
    if key not in _CACHE:
        _CACHE[key] = _build(scale_val)
    nc = _CACHE[key]

    bias_pack = _pack_bias(bias)
    in_maps = [
        {"x": np.ascontiguousarray(x[c * BLOC : (c + 1) * BLOC]),
         "h": h2b, "bias": bias_pack}
        for c in range(NCORES)
    ]
    res = run_bass_kernel_spmd(nc, in_maps, list(range(NCORES)),
                               trace=PROFILE)
    LAST_RESULT = res
    out = np.concatenate([res.results[c]["out"] for c in range(NCORES)],
                         axis=0)
    return out



# revision 10
# speedup vs baseline: 2.1141x; 2.1141x over previous
"""Bass/Trainium2 kernel for nn_HadamardClassifier.

Math: out = -scale * l2norm(x) @ H + bias, with H = H_16384[:2048, :14951]
(Sylvester). Since H_16384 = H_8 (x) H_2048 and rows < 2048 hit only row 0 of
the H_8 factor (all +1), H is just H_2048 tiled horizontally:
    out[:, j] = (x * (-scale/||x||_2)) @ H_2048[:, j % 2048] + bias[j]

Sharding: batch-parallel across 8 cores (512 rows each).

Precision budget (tolerance is max-abs-err / max|out| < 2e-2):
  - H_2048 entries +-1 are stored fp8_e4m3 (exact, halves the H load).
  - x is normalized in f32; the per-row -scale/||x|| factor rides the PE
    transpose as a diagonal moving operand; xT is kept bf16.  The PE
    accepts mixed bf16 x fp8 operands (verified exact on HW).
  - Z and the output are bf16 (~0.2% of element each); host upcasts.

Schedule: per row-chunk cb, phase1 (load/normalize/transpose) then
phase2 (matmul with one LDWEIGHTS per 4 matmuls via js-inner order,
bias adds, one contiguous 3.5 MiB store) emitted back-to-back so the
PE never idles long enough for the HAM clock gate to re-throttle.
"""

import math

import numpy as np

B, IN, OUT = 4096, 2048, 14951
NCORES = 8
BLOC = B // NCORES  # 512
P = 128
PERIOD = 2048
NFULL = 7  # blocks covered by the contiguous main store
TAIL = OUT - NFULL * PERIOD  # 615 = 512 (js0 blk7) + 103 (js1 blk7)
EPS = 1e-12
NCB = BLOC // P  # 4 batch chunks per core
NIC = IN // P  # 16 contraction chunks
NJS = PERIOD // 512  # 4 column slabs of 512
# bias packed js-major: per js, its 512-col piece of every block, 512-padded
NBLK_JS = [8, 8, 7, 7]  # js1's 8th seg is the 103-wide tail (padded)
OFF_JS = [0, 4096, 8192, 11776]
BIAS_PACK = 15360

_CACHE = {}
LAST_RESULT = None
PROFILE = False


def _build(scale_val: float):
    from contextlib import ExitStack

    import concourse.bass as bass
    import concourse.mybir as mybir
    import concourse.tile as tile
    from concourse import bacc, masks

    f32 = mybir.dt.float32
    bf16 = mybir.dt.bfloat16
    fp8 = mybir.dt.float8e4
    nc = bacc.Bacc("TRN2", target_bir_lowering=False, debug=False,
                   num_devices=NCORES)

    x_d = nc.dram_tensor("x", [BLOC, IN], f32, kind="ExternalInput")
    h_d = nc.dram_tensor("h", [IN, PERIOD], fp8, kind="ExternalInput")
    b_d = nc.dram_tensor("bias", [1, BIAS_PACK], bf16, kind="ExternalInput")
    o_d = nc.dram_tensor("out", [BLOC, OUT], bf16, kind="ExternalOutput")

    # [2048 rows] -> [p, ic] view so each SBUF partition p holds rows ic*128+p
    h_v = h_d[:, :].rearrange("(ic p) j -> p ic j", p=P)

    with tile.TileContext(nc) as tc, ExitStack() as ctx:
        p_const = ctx.enter_context(tc.tile_pool(name="const", bufs=1))
        p_x = ctx.enter_context(tc.tile_pool(name="xload", bufs=NCB))
        p_w = ctx.enter_context(tc.tile_pool(name="work", bufs=1))
        p_ss = ctx.enter_context(tc.tile_pool(name="small", bufs=16))
        p_xq = ctx.enter_context(tc.tile_pool(name="xq", bufs=NCB))
        p_z = ctx.enter_context(tc.tile_pool(name="zsb", bufs=4))
        p_o = ctx.enter_context(tc.tile_pool(name="ostage", bufs=2))
        p_tl = ctx.enter_context(tc.tile_pool(name="tail", bufs=NCB))
        p_pst = ctx.enter_context(
            tc.tile_pool(name="psum_t", bufs=2, space="PSUM"))
        p_psz = ctx.enter_context(
            tc.tile_pool(name="psum_z", bufs=6, space="PSUM"))

        ident = p_const.tile([P, P], f32, tag="ident")
        masks.make_identity(nc, ident[:])

        # HAM warmup: open the PE clock gate before the real stream starts
        warm = p_pst.tile([P, 512], f32, tag="pst")
        for _ in range(24):
            nc.tensor.matmul(warm[:, 0:P], ident[:], ident[:], start=True,
                             stop=True)

        # all loads ride the sync ring, interleaved so x(cb) chunks land
        # just ahead of their phase-1 use while H slabs fill the gaps;
        # the scalar ring (ACT queue) stays free for phase-1 compute
        bias_rep = p_const.tile([P, BIAS_PACK], bf16, tag="bias_rep")
        nc.sync.dma_start(out=bias_rep[0:1, :], in_=b_d[:, :])
        hs = p_const.tile([P, NIC, PERIOD], fp8, tag="hs")
        xnats = []
        for cb in range(NCB):
            xnat = p_x.tile([P, IN], f32, tag="xnat", name=f"xnat{cb}")
            nc.sync.dma_start(out=xnat[:], in_=x_d[cb * P : (cb + 1) * P, :])
            xnats.append(xnat)
            nc.sync.dma_start(out=hs[:, cb * 4 : (cb + 1) * 4, :],
                              in_=h_v[:, cb * 4 : (cb + 1) * 4, :])
        # bias broadcast on gpsimd: runs during phase 1 while the DVE is
        # mostly idle (the shared SBUF port only contends with 2-port DVE ops)
        for a in range(0, BIAS_PACK, 2048):
            w = min(2048, BIAS_PACK - a)
            nc.gpsimd.partition_broadcast(bias_rep[:, a : a + w],
                                          bias_rep[0:1, a : a + w])

        tails = []

        def phase1(cb):
            xnat = xnats[cb]
            sq = p_w.tile([P, IN], bf16, tag="work")
            ss = p_ss.tile([P, 1], f32, tag="ss")
            nc.scalar.activation(sq[:], xnat[:],
                                 mybir.ActivationFunctionType.Square,
                                 accum_out=ss[:])
            nc.vector.tensor_scalar_max(ss[:], ss[:], EPS)
            nrm = p_ss.tile([P, 1], f32, tag="nrm")
            nc.scalar.sqrt(nrm[:], ss[:])
            inv = p_ss.tile([P, 1], f32, tag="inv")
            nc.vector.reciprocal(inv[:], nrm[:])
            mult = p_ss.tile([P, 1], f32, tag="mult")
            nc.vector.tensor_scalar_mul(mult[:], inv[:], -scale_val)
            # rows *= -scale/||row|| in place on ACT (per-partition scale)
            nc.scalar.mul(xnat[:], xnat[:], mult[:, 0:1])

            xq = p_xq.tile([P, NIC, P], bf16, tag="xq")
            for g in range(4):  # groups of 4 transposes share one psum tile
                pst = p_pst.tile([P, 512], f32, tag="pst")
                for i in range(4):
                    ic = g * 4 + i
                    nc.tensor.transpose(pst[:, i * P : (i + 1) * P],
                                        xnat[:, ic * P : (ic + 1) * P],
                                        ident[:])
                dst = xq[:, g * 4 : (g + 1) * 4, :].rearrange(
                    "p i c -> p (i c)")
                if g % 2 == 0:
                    nc.scalar.copy(dst, pst[:])
                else:
                    nc.vector.tensor_copy(dst, pst[:])
            return xq

        def phase2(cb, xq):
            r0 = cb * P
            psz = [p_psz.tile([P, 512], f32, tag="psz", name=f"psz{js}")
                   for js in range(NJS)]
            for ic in range(NIC):
                for js in range(NJS):
                    nc.tensor.matmul(psz[js][:], xq[:, ic, :],
                                     hs[:, ic, js * 512 : (js + 1) * 512],
                                     start=(ic == 0), stop=(ic == NIC - 1))
            ost = p_o.tile([P, NFULL, PERIOD], bf16, tag="ostage")
            for js in range(NJS):
                c0 = js * 512
                boff = OFF_JS[js]
                zsb = p_z.tile([P, 512], bf16, tag="zsb")
                if js % 2 == 0:
                    nc.scalar.copy(zsb[:], psz[js][:])
                else:
                    nc.vector.tensor_copy(zsb[:], psz[js][:])

                zb4 = zsb[:, :].unsqueeze(1).broadcast_to((P, 4, 512))
                zb3 = zsb[:, :].unsqueeze(1).broadcast_to((P, 3, 512))
                nc.vector.tensor_add(
                    ost[:, 0:4, c0 : c0 + 512], zb4,
                    bias_rep[:, boff : boff + 2048].rearrange(
                        "p (b c) -> p b c", b=4))
                nc.vector.tensor_add(
                    ost[:, 4:7, c0 : c0 + 512], zb3,
                    bias_rep[:, boff + 2048 : boff + 3584].rearrange(
                        "p (b c) -> p b c", b=3))
                if js == 0:
                    tl = p_tl.tile([P, TAIL], bf16, tag="tail")
                    tails.append(tl)
                    nc.vector.tensor_add(
                        tl[:, 0:512], zsb[:, :],
                        bias_rep[:, boff + 3584 : boff + 4096])
                elif js == 1:
                    tl = tails[cb]
                    nc.vector.tensor_add(
                        tl[:, 512:615], zsb[:, 0:103],
                        bias_rep[:, boff + 3584 : boff + 3687])
                    eng = nc.sync if cb % 2 else nc.scalar
                    eng.dma_start(
                        out=o_d[r0 : r0 + P, NFULL * PERIOD : OUT],
                        in_=tl[:, :])
            # one contiguous 28 KB/partition store for blocks 0..6
            eng = nc.scalar if cb % 2 else nc.sync
            eng.dma_start(
                out=o_d[r0 : r0 + P, 0 : NFULL * PERIOD],
                in_=ost[:, :, :].rearrange("p b c -> p (b c)"))

        # software pipeline: phase1(cb+1) is emitted before phase2(cb) so the
        # PE queue alternates transpose and matmul blocks with no dead gaps
        # (a >3.4us PE idle re-throttles the HAM clock gate to half rate)
        xq_prev = phase1(0)
        for cb in range(1, NCB):
            xq_cur = phase1(cb)
            phase2(cb - 1, xq_prev)
            xq_prev = xq_cur
        phase2(NCB - 1, xq_prev)

    nc.compile()
    return nc


def _pack_bias(bias: np.ndarray) -> np.ndarray:
    import ml_dtypes
    pack = np.zeros((1, BIAS_PACK), dtype=np.float32)
    for js in range(NJS):
        for blk in range(NBLK_JS[js]):
            src0 = blk * PERIOD + js * 512
            seg = bias[src0 : src0 + 512]
            pack[0, OFF_JS[js] + blk * 512 :
                 OFF_JS[js] + blk * 512 + len(seg)] = seg
    return pack.astype(ml_dtypes.bfloat16)


def kernel(x, hadamard, scale, bias):
    global LAST_RESULT
    import ml_dtypes
    from concourse.bass_utils import run_bass_kernel_spmd

    x = np.ascontiguousarray(np.asarray(x, dtype=np.float32))
    hadamard = np.asarray(hadamard, dtype=np.float32)
    bias = np.asarray(bias, dtype=np.float32)
    scale_val = float(np.asarray(scale).reshape(-1)[0])

    h2 = np.ascontiguousarray(hadamard[:, :PERIOD])
    # the whole kernel rests on the 2048-periodicity of the weight columns
    for k in range(1, NFULL):
        assert np.array_equal(hadamard[:, k * PERIOD : (k + 1) * PERIOD], h2), (
            "hadamard is not 2048-periodic; kernel assumption violated")
    assert np.array_equal(hadamard[:, NFULL * PERIOD :], h2[:, :TAIL])
    h8 = h2.astype(ml_dtypes.float8_e4m3)
    assert np.array_equal(h8.astype(np.float32), h2), "H not fp8-exact"

    key = scale_val
    if key not in _CACHE:
        _CACHE[key] = _build(scale_val)
    nc = _CACHE[key]

    bias_pack = _pack_bias(bias)
    in_maps = [
        {"x": np.ascontiguousarray(x[c * BLOC : (c + 1) * BLOC]),
         "h": h8, "bias": bias_pack}
        for c in range(NCORES)
    ]
    res = run_bass_kernel_spmd(nc, in_maps, list(range(NCORES)),
                               trace=PROFILE)
    LAST_RESULT = res
    out = np.concatenate(
        [res.results[c]["out"].astype(np.float32) for c in range(NCORES)],
        axis=0)
    return out


# revision 13
# speedup vs baseline: 2.2979x; 1.0869x over previous
"""Bass/Trainium2 kernel for nn_HadamardClassifier.

Math: out = -scale * l2norm(x) @ H + bias, with H = H_16384[:2048, :14951]
(Sylvester). Since H_16384 = H_8 (x) H_2048 and rows < 2048 hit only row 0 of
the H_8 factor (all +1), H is just H_2048 tiled horizontally:
    out[:, j] = (x * (-scale/||x||_2)) @ H_2048[:, j % 2048] + bias[j]

Sharding: batch-parallel across 8 cores (512 rows each).

Precision budget (tolerance is max-abs-err / max|out| < 2e-2):
  - H_2048 entries +-1 are stored fp8_e4m3 (exact, halves the H load).
  - x is normalized in f32; the per-row -scale/||x|| factor rides the PE
    transpose as a diagonal moving operand; xT is kept bf16.  The PE
    accepts mixed bf16 x fp8 operands (verified exact on HW).
  - Z and the output are bf16 (~0.2% of element each); host upcasts.

Schedule: per row-chunk cb, phase1 (load/normalize/transpose) then
phase2 (matmul with one LDWEIGHTS per 4 matmuls via js-inner order,
bias adds, one contiguous 3.5 MiB store) emitted back-to-back so the
PE never idles long enough for the HAM clock gate to re-throttle.
"""

import math

import numpy as np

B, IN, OUT = 4096, 2048, 14951
NCORES = 8
BLOC = B // NCORES  # 512
P = 128
PERIOD = 2048
NFULL = 7  # blocks covered by the contiguous main store
TAIL = OUT - NFULL * PERIOD  # 615 = 512 (js0 blk7) + 103 (js1 blk7)
EPS = 1e-12
NCB = BLOC // P  # 4 batch chunks per core
NIC = IN // P  # 16 contraction chunks
NJS = PERIOD // 512  # 4 column slabs of 512
# bias packed js-major: per js, its 512-col piece of every block, 512-padded
NBLK_JS = [8, 8, 7, 7]  # js1's 8th seg is the 103-wide tail (padded)
OFF_JS = [0, 4096, 8192, 11776]
BIAS_PACK = 15360

_CACHE = {}
LAST_RESULT = None
PROFILE = False


def _build(scale_val: float):
    from contextlib import ExitStack

    import concourse.bass as bass
    import concourse.mybir as mybir
    import concourse.tile as tile
    from concourse import bacc, masks

    f32 = mybir.dt.float32
    bf16 = mybir.dt.bfloat16
    fp8 = mybir.dt.float8e4
    nc = bacc.Bacc("TRN2", target_bir_lowering=False, debug=False,
                   num_devices=NCORES)

    x_d = nc.dram_tensor("x", [BLOC, IN], f32, kind="ExternalInput")
    h_d = nc.dram_tensor("h", [IN, PERIOD], fp8, kind="ExternalInput")
    b_d = nc.dram_tensor("bias", [1, BIAS_PACK], bf16, kind="ExternalInput")
    o_d = nc.dram_tensor("out", [BLOC, OUT], bf16, kind="ExternalOutput")

    # [2048 rows] -> [p, ic] view so each SBUF partition p holds rows ic*128+p
    h_v = h_d[:, :].rearrange("(ic p) j -> p ic j", p=P)

    with tile.TileContext(nc) as tc, ExitStack() as ctx:
        p_const = ctx.enter_context(tc.tile_pool(name="const", bufs=1))
        p_x = ctx.enter_context(tc.tile_pool(name="xload", bufs=NCB))
        p_w = ctx.enter_context(tc.tile_pool(name="work", bufs=1))
        p_ss = ctx.enter_context(tc.tile_pool(name="small", bufs=16))
        p_xq = ctx.enter_context(tc.tile_pool(name="xq", bufs=NCB))
        p_z = ctx.enter_context(tc.tile_pool(name="zsb", bufs=4))
        p_o = ctx.enter_context(tc.tile_pool(name="ostage", bufs=2))
        p_tl = ctx.enter_context(tc.tile_pool(name="tail", bufs=NCB))
        p_pst = ctx.enter_context(
            tc.tile_pool(name="psum_t", bufs=2, space="PSUM"))
        p_psz = ctx.enter_context(
            tc.tile_pool(name="psum_z", bufs=6, space="PSUM"))

        ident = p_const.tile([P, P], f32, tag="ident")
        masks.make_identity(nc, ident[:])

        # HAM warmup: open the PE clock gate before the real stream starts
        warm = p_pst.tile([P, 512], f32, tag="pst")
        for _ in range(24):
            nc.tensor.matmul(warm[:, 0:P], ident[:], ident[:], start=True,
                             stop=True)

        # all loads ride the sync ring, interleaved so x(cb) chunks land
        # just ahead of their phase-1 use while H slabs fill the gaps;
        # the scalar ring (ACT queue) stays free for phase-1 compute
        bias_rep = p_const.tile([P, BIAS_PACK], bf16, tag="bias_rep")
        nc.sync.dma_start(out=bias_rep[0:1, :], in_=b_d[:, :])
        hs = p_const.tile([P, NIC, PERIOD], fp8, tag="hs")
        xnats = []
        for cb in range(NCB):
            xnat = p_x.tile([P, IN], f32, tag="xnat", name=f"xnat{cb}")
            nc.sync.dma_start(out=xnat[:], in_=x_d[cb * P : (cb + 1) * P, :])
            xnats.append(xnat)
            nc.sync.dma_start(out=hs[:, cb * 4 : (cb + 1) * 4, :],
                              in_=h_v[:, cb * 4 : (cb + 1) * 4, :])
        # bias broadcast on gpsimd: runs during phase 1 while the DVE is
        # mostly idle (the shared SBUF port only contends with 2-port DVE ops)
        for a in range(0, BIAS_PACK, 2048):
            w = min(2048, BIAS_PACK - a)
            nc.gpsimd.partition_broadcast(bias_rep[:, a : a + w],
                                          bias_rep[0:1, a : a + w])

        tails = []

        def phase1(cb):
            # transpose the RAW rows straight off the DMA; the l2 norm is
            # computed in parallel and -scale/||row|| is applied later, on
            # the per-partition scale input of the PSUM->SBUF copies
            xnat = xnats[cb]
            xq = p_xq.tile([P, NIC, P], bf16, tag="xq")
            for g in range(4):  # groups of 4 transposes share one psum tile
                pst = p_pst.tile([P, 512], f32, tag="pst")
                for i in range(4):
                    ic = g * 4 + i
                    nc.tensor.transpose(pst[:, i * P : (i + 1) * P],
                                        xnat[:, ic * P : (ic + 1) * P],
                                        ident[:])
                dst = xq[:, g * 4 : (g + 1) * 4, :].rearrange(
                    "p i c -> p (i c)")
                if g % 2 == 0:
                    nc.scalar.copy(dst, pst[:])
                else:
                    nc.vector.tensor_copy(dst, pst[:])

            sq = p_w.tile([P, IN], bf16, tag="work")
            ss = p_ss.tile([P, 1], f32, tag="ss")
            nc.scalar.activation(sq[:], xnat[:],
                                 mybir.ActivationFunctionType.Square,
                                 accum_out=ss[:])
            nc.vector.tensor_scalar_max(ss[:], ss[:], EPS)
            nrm = p_ss.tile([P, 1], f32, tag="nrm")
            nc.scalar.sqrt(nrm[:], ss[:])
            inv = p_ss.tile([P, 1], f32, tag="inv")
            nc.vector.reciprocal(inv[:], nrm[:])
            mult = p_ss.tile([P, 1], f32, tag="mult")
            nc.vector.tensor_scalar_mul(mult[:], inv[:], -scale_val)
            return xq, mult

        def phase2(cb, xq, mult):
            r0 = cb * P
            psz = [p_psz.tile([P, 512], f32, tag="psz", name=f"psz{js}")
                   for js in range(NJS)]
            if cb < NCB - 1:
                # ic-outer: spreads H consumption so the matmuls never wait
                # on the H load; drains overlap the next cb's PE blocks
                for ic in range(NIC):
                    for js in range(NJS):
                        nc.tensor.matmul(psz[js][:], xq[:, ic, :],
                                         hs[:, ic, js * 512 : (js + 1) * 512],
                                         start=(ic == 0), stop=(ic == NIC - 1))
            else:
                # last chunk: js-outer so each slab drains while the next
                # slab's matmuls run (nothing follows to hide the drain)
                for js in range(NJS):
                    for ic in range(NIC):
                        nc.tensor.matmul(psz[js][:], xq[:, ic, :],
                                         hs[:, ic, js * 512 : (js + 1) * 512],
                                         start=(ic == 0), stop=(ic == NIC - 1))
            ost = p_o.tile([P, NFULL, PERIOD], bf16, tag="ostage")
            for js in range(NJS):
                c0 = js * 512
                boff = OFF_JS[js]
                zsb = p_z.tile([P, 512], bf16, tag="zsb")
                if js % 2 == 0:
                    nc.scalar.mul(zsb[:], psz[js][:], mult[:, 0:1])
                else:
                    nc.vector.tensor_scalar_mul(zsb[:], psz[js][:],
                                                mult[:, 0:1])

                zb4 = zsb[:, :].unsqueeze(1).broadcast_to((P, 4, 512))
                zb3 = zsb[:, :].unsqueeze(1).broadcast_to((P, 3, 512))
                nc.vector.tensor_add(
                    ost[:, 0:4, c0 : c0 + 512], zb4,
                    bias_rep[:, boff : boff + 2048].rearrange(
                        "p (b c) -> p b c", b=4))
                nc.vector.tensor_add(
                    ost[:, 4:7, c0 : c0 + 512], zb3,
                    bias_rep[:, boff + 2048 : boff + 3584].rearrange(
                        "p (b c) -> p b c", b=3))
                if js == 0:
                    tl = p_tl.tile([P, TAIL], bf16, tag="tail")
                    tails.append(tl)
                    nc.vector.tensor_add(
                        tl[:, 0:512], zsb[:, :],
                        bias_rep[:, boff + 3584 : boff + 4096])
                elif js == 1:
                    tl = tails[cb]
                    nc.vector.tensor_add(
                        tl[:, 512:615], zsb[:, 0:103],
                        bias_rep[:, boff + 3584 : boff + 3687])
                    eng = nc.sync if cb % 2 else nc.scalar
                    eng.dma_start(
                        out=o_d[r0 : r0 + P, NFULL * PERIOD : OUT],
                        in_=tl[:, :])
            # one contiguous 28 KB/partition store for blocks 0..6
            eng = nc.scalar if cb % 2 else nc.sync
            eng.dma_start(
                out=o_d[r0 : r0 + P, 0 : NFULL * PERIOD],
                in_=ost[:, :, :].rearrange("p b c -> p (b c)"))

        # software pipeline: phase1(cb+1) is emitted before phase2(cb) so the
        # PE queue alternates transpose and matmul blocks with no dead gaps
        # (a >3.4us PE idle re-throttles the HAM clock gate to half rate)
        prev = phase1(0)
        for cb in range(1, NCB):
            cur = phase1(cb)
            phase2(cb - 1, *prev)
            prev = cur
        phase2(NCB - 1, *prev)

    nc.compile()
    return nc


def _pack_bias(bias: np.ndarray) -> np.ndarray:
    import ml_dtypes
    pack = np.zeros((1, BIAS_PACK), dtype=np.float32)
    for js in range(NJS):
        for blk in range(NBLK_JS[js]):
            src0 = blk * PERIOD + js * 512
            seg = bias[src0 : src0 + 512]
            pack[0, OFF_JS[js] + blk * 512 :
                 OFF_JS[js] + blk * 512 + len(seg)] = seg
    return pack.astype(ml_dtypes.bfloat16)


def kernel(x, hadamard, scale, bias):
    global LAST_RESULT
    import ml_dtypes
    from concourse.bass_utils import run_bass_kernel_spmd

    x = np.ascontiguousarray(np.asarray(x, dtype=np.float32))
    hadamard = np.asarray(hadamard, dtype=np.float32)
    bias = np.asarray(bias, dtype=np.float32)
    scale_val = float(np.asarray(scale).reshape(-1)[0])

    h2 = np.ascontiguousarray(hadamard[:, :PERIOD])
    # the whole kernel rests on the 2048-periodicity of the weight columns
    for k in range(1, NFULL):
        assert np.array_equal(hadamard[:, k * PERIOD : (k + 1) * PERIOD], h2), (
            "hadamard is not 2048-periodic; kernel assumption violated")
    assert np.array_equal(hadamard[:, NFULL * PERIOD :], h2[:, :TAIL])
    h8 = h2.astype(ml_dtypes.float8_e4m3)
    assert np.array_equal(h8.astype(np.float32), h2), "H not fp8-exact"

    key = scale_val
    if key not in _CACHE:
        _CACHE[key] = _build(scale_val)
    nc = _CACHE[key]

    bias_pack = _pack_bias(bias)
    in_maps = [
        {"x": np.ascontiguousarray(x[c * BLOC : (c + 1) * BLOC]),
         "h": h8, "bias": bias_pack}
        for c in range(NCORES)
    ]
    res = run_bass_kernel_spmd(nc, in_maps, list(range(NCORES)),
                               trace=PROFILE)
    LAST_RESULT = res
    out = np.concatenate(
        [res.results[c]["out"].astype(np.float32) for c in range(NCORES)],
        axis=0)
    return out
